# revision 22
# baseline (speedup 1.0000x reference)
"""Trainium2 (8 NeuronCores, SPMD) kernel for a 4-layer GCN + mean-pool + FC head.

Strategy (nodes dst-sharded contiguously across 8 cores; edges binned by
(dst-window of 128, src-chunk of 25000) so int16 gather indices work):

Per conv layer (one SPMD launch, same compiled program for all 4 layers):
  xt = dinv * x (bf16, node-major, full replica in each core's HBM)
  M[:, d]  = sum_{e: dst[e]=d} xt[:, src[e]] + 2*xt[:, d]
  xo[:, d] = relu(W^T (dinv[d] * M[:, d]) + b)          (feature-major bf16)

Device mechanics (ring-buffered streaming):
  - Edge slots are packed 16-granular per (window, chunk) bin into four
    per-chunk streams; dma_gather calls (SWDGE) cut the streams at 1024-idx
    boundaries (minimizing the ~374ns/call fixed cost + 2.3ns/idx Q7 cost,
    which is the kernel's bottleneck engine).
  - Gather destinations are ring tiles [128, 8, 128]; window consumers read
    tile columns through a trace-time map, so the Tile framework pipelines
    gathers ~20 calls ahead of the PE.
  - S one-hot tiles are built per (window, tile) incidence on DVE
    (is_equal vs iota, sentinel dstloc for out-of-bin slots), batched 8 per
    op into ring tiles.
  - PE accumulates M per window in PSUM via full-128-k matmuls (S masks
    pad/foreign slots), a 2I matmul adds the self loop, DVE scales by dinv,
    PE applies W, ACT applies bias+ReLU.

Host (numpy): deg/dinv, edge binning, inter-layer transpose + dinv scale,
final mean-pool (segment reduceat over sorted batch) and the tiny FC head.
"""
import contextlib
import ctypes
import sys
import types

import numpy as np
import ml_dtypes

import concourse.bass as bass
import concourse.bacc as bacc
import concourse.mybir as mybir
import concourse.tile as tile

BF16 = mybir.dt.bfloat16
F32 = mybir.dt.float32
I16 = mybir.dt.int16
AF = mybir.ActivationFunctionType
NPBF16 = ml_dtypes.bfloat16

P = 128
PAD_DSTLOC = 200.0  # sentinel dst-local id for padding edges (is_equal -> 0)

N_NODES = 100000
N_CORES = 8
N_CONVS = 4
CHUNKS = 4
SBATCH = 8  # incidences per is_equal op / per S ring tile
NQUEUES = 4  # SWDGE queues for gathers
CALL = 1024  # idxs per dma_gather call (ucode cap)
BUFS_G = 28
BUFS_S = 22
BUFS_IDX = 10
LOOKAHEAD = 3  # windows of gather prefetch

NPC = N_NODES // N_CORES  # 12500
NWIN = (NPC + P - 1) // P  # 98
NPC_PAD = NWIN * P  # 12544
CHR = N_NODES // CHUNKS  # 25000


# ---------------------------------------------------------------------------
# axon NTFF profile hook (this image's antenv lacks axon_hooks; recreate it so
# run_bass_kernel_spmd(trace=True) can report HW exec time)
# ---------------------------------------------------------------------------
def _install_profile_shim():
    if "antenv.axon_hooks" in sys.modules:
        return
    so_path = "/opt/axon/libaxon_pjrt.so"

    def _ntff_profile_via_ctypes(path):
        try:
            lib = ctypes.CDLL(path)
        except OSError:
            return None
        if not hasattr(lib, "axon_start_nrt_profile"):
            return None
        lib.axon_start_nrt_profile.argtypes = [
            ctypes.POINTER(ctypes.c_int64),
            ctypes.c_size_t,
        ]
        lib.axon_start_nrt_profile.restype = ctypes.c_int64
        lib.axon_stop_nrt_profile.argtypes = [ctypes.c_char_p]
        lib.axon_stop_nrt_profile.restype = ctypes.c_int64

        @contextlib.contextmanager
        def _hook(output_dir, device_ids):
            import jax

            jax.devices()
            if device_ids:
                ids = (ctypes.c_int64 * len(device_ids))(*device_ids)
                rc = lib.axon_start_nrt_profile(ids, len(device_ids))
            else:
                rc = lib.axon_start_nrt_profile(None, 0)
            if rc != 0:
                raise RuntimeError(f"axon_start_nrt_profile rc={rc}")
            try:
                yield
            finally:
                n = lib.axon_stop_nrt_profile(str(output_dir).encode())
                if n < 0:
                    raise RuntimeError(f"axon_stop_nrt_profile rc={n}")

        return _hook

    mod = types.ModuleType("antenv.axon_hooks")
    hook = _ntff_profile_via_ctypes(so_path)
    mod.get_axon_ntff_profile_hook = lambda: hook
    mod.set_axon_ntff_profile_hook = lambda h: None
    try:
        import antenv

        antenv.axon_hooks = mod
    except ImportError:
        pass
    sys.modules["antenv.axon_hooks"] = mod


_install_profile_shim()

from concourse.bass_utils import run_bass_kernel_spmd  # noqa: E402


# ---------------------------------------------------------------------------
# host-side edge preprocessing
# ---------------------------------------------------------------------------
def _balance_dsts(dst, chunk):
    """Greedy dst->virtual-position assignment balancing (window, chunk) bin
    counts across the 8 cores (the shared SPMD program pads every bin to the
    max over cores, so balance directly cuts gathered pad slots ~4%)."""
    deg = np.bincount(dst, minlength=N_NODES)
    order = np.argsort(-deg, kind="stable")
    cnt_c = np.zeros((N_NODES, CHUNKS), np.int64)
    np.add.at(cnt_c, (dst, chunk), 1)
    assign = np.full(N_NODES, -1, np.int64)
    used = 0
    for s in range(NWIN):
        cap = P if s < NWIN - 1 else NPC - P * (NWIN - 1)
        nodes = order[used : used + cap * N_CORES]
        used += len(nodes)
        load = np.zeros((N_CORES, CHUNKS), np.int64)
        fill = np.zeros(N_CORES, np.int64)
        for nd in nodes:
            v = cnt_c[nd]
            best, bestcost = -1, None
            for c in range(N_CORES):
                if fill[c] >= cap:
                    continue
                nl = load[c] + v
                cost = (int(nl.max()), int(nl.sum()))
                if bestcost is None or cost < bestcost:
                    best, bestcost = c, cost
            load[best] += v
            assign[nd] = best * NPC + s * P + fill[best]
            fill[best] += 1
    assert (assign >= 0).all() and used == N_NODES
    return assign


def _host_prep(src, dst):
    """Bin edges by (core, window, chunk) at 16-slot granularity.

    Returns the shared program structure (bin sizes are the max over cores so
    one SPMD program fits all cores) and per-core idx/dstloc arrays. Padding
    slots use idx 0 with a sentinel dst-local id that zeroes their S column.
    """
    chunk = src // CHR
    srcloc = (src - chunk * CHR).astype(np.int64)
    assert srcloc.max() < 32768

    assign = _balance_dsts(dst, chunk)  # dst node -> virtual position
    vdst = assign[dst]
    core = vdst // NPC
    drem = vdst % NPC
    win = drem // P
    dloc = drem - win * P

    binid = ((core * NWIN + win) * CHUNKS + chunk).astype(np.int64)
    counts = np.bincount(binid, minlength=N_CORES * NWIN * CHUNKS).reshape(
        N_CORES, NWIN, CHUNKS
    )
    mx = counts.max(axis=0)
    # bins pack at raw max-over-cores size: gather calls cut the stream at
    # 1024-slot boundaries, so bins need no alignment of their own
    sz16 = mx.astype(np.int64)

    # per-chunk streams: window-ordered bins back to back; stream padded x128
    clen = sz16.sum(axis=0)  # [CHUNKS]
    clen128 = -(-clen // P) * P
    chunk_base = np.concatenate([[0], np.cumsum(clen128)])
    bin_start = np.zeros((NWIN, CHUNKS), np.int64)
    for c in range(CHUNKS):
        bin_start[:, c] = chunk_base[c] + np.concatenate(
            [[0], np.cumsum(sz16[:, c])[:-1]]
        )
    L = int(chunk_base[-1])
    nt_total = L // P

    # gather calls: cut each chunk stream at 1024-slot boundaries
    calls = []  # (chunk, slot0, nidx)
    for c in range(CHUNKS):
        s0 = 0
        while s0 < clen128[c]:
            nidx = int(min(CALL, clen128[c] - s0))
            calls.append((c, int(chunk_base[c] + s0), nidx))
            s0 += nidx

    # incidences: (window, chunk) x overlapped tile
    inc_map = [[[] for _ in range(CHUNKS)] for _ in range(NWIN)]
    inc_ranges = []  # per incidence: (t, a, b) absolute slot clip range
    ni = 0
    for w in range(NWIN):
        for c in range(CHUNKS):
            if sz16[w, c] == 0:
                continue
            a = int(bin_start[w, c])
            b = a + int(sz16[w, c])
            for t in range(a // P, -(-b // P)):
                inc_map[w][c].append((t, ni))
                inc_ranges.append((t, max(a, t * P), min(b, (t + 1) * P)))
                ni += 1

    order = np.argsort(binid, kind="stable")
    sorted_bin = binid[order]
    uniq, first_idx = np.unique(sorted_bin, return_index=True)
    start_of_bin = np.zeros(N_CORES * NWIN * CHUNKS, np.int64)
    start_of_bin[uniq] = first_idx
    within = np.arange(len(order)) - start_of_bin[sorted_bin]

    s_core = core[order]
    pos = bin_start[win[order], chunk[order]] + within

    per_core = []
    for cc in range(N_CORES):
        m = s_core == cc
        idx_arr = np.zeros(L, np.int64)
        dst_arr = np.full(L, PAD_DSTLOC, np.float32)
        idx_arr[pos[m]] = srcloc[order][m]
        dst_arr[pos[m]] = dloc[order][m]
        idx16 = idx_arr.reshape(L // 16, 16).T.astype(np.int16)  # [16, L//16]
        idx128 = np.tile(idx16, (8, 1))
        # window-masked dstloc per incidence
        dinc = np.full((P, ni), PAD_DSTLOC, np.float32)
        for i, (t, a, b) in enumerate(inc_ranges):
            dinc[a - t * P : b - t * P, i] = dst_arr[a:b]
        per_core.append({"idx": idx128, "dstloc": dinc.astype(NPBF16)})
    return sz16, calls, inc_map, nt_total, ni, per_core, assign


# ---------------------------------------------------------------------------
# device program (one conv layer; same program reused for all 4 launches)
# ---------------------------------------------------------------------------
def _build_program(sz16, calls, inc_map, nt_total, n_inc):
    NT = nt_total
    L = NT * P
    # call list per chunk for issue ordering
    calls_by_chunk = [[] for _ in range(CHUNKS)]
    for c, s0, nidx in calls:
        calls_by_chunk[c].append((s0, nidx))

    nc = bacc.Bacc(
        "TRN2", target_bir_lowering=False, debug=False, num_swdge_queues=NQUEUES
    )
    xt = nc.dram_tensor("xt", [N_NODES, P], BF16, kind="ExternalInput")
    xt_own = nc.dram_tensor("xt_own", [NPC_PAD, P], BF16, kind="ExternalInput")
    idx_in = nc.dram_tensor("idx", [P, L // 16], I16, kind="ExternalInput")
    dstloc_in = nc.dram_tensor("dstloc", [P, n_inc], BF16, kind="ExternalInput")
    w_in = nc.dram_tensor("wmat", [P, P], BF16, kind="ExternalInput")
    b_in = nc.dram_tensor("bias", [P, 1], F32, kind="ExternalInput")
    dinvr_in = nc.dram_tensor("dinv_row", [P, NPC_PAD], BF16, kind="ExternalInput")
    iota_in = nc.dram_tensor("iota", [P, P], BF16, kind="ExternalInput")
    s2i_in = nc.dram_tensor("s2i", [P, P], BF16, kind="ExternalInput")
    xo = nc.dram_tensor("xo", [P, NPC_PAD], BF16, kind="ExternalOutput")

    with tile.TileContext(nc) as tc:
        with (
            tc.tile_pool(name="const", bufs=1) as cpool,
            tc.tile_pool(name="idx", bufs=BUFS_IDX) as ipool,
            tc.tile_pool(name="g", bufs=BUFS_G) as gpool,
            tc.tile_pool(name="s", bufs=BUFS_S) as spool,
            tc.tile_pool(name="selfp", bufs=4) as selfpool,
            tc.tile_pool(name="m", bufs=3) as mpool,
            tc.tile_pool(name="xop", bufs=3) as xopool,
            tc.tile_pool(name="psm", bufs=4, space="PSUM") as psm_pool,
            tc.tile_pool(name="psh", bufs=2, space="PSUM") as psh_pool,
        ):
            const_tiles = {}

            def load_consts():
                # emitted after the first gather so the opening dma_gather
                # isn't stuck behind ~4MB of constant DMAs on the HWDGE FIFO
                if const_tiles:
                    return
                dst_t = cpool.tile([P, n_inc], BF16)
                nc.sync.dma_start(dst_t[:], dstloc_in[:])
                dinvr_t = cpool.tile([P, NPC_PAD], BF16)
                nc.sync.dma_start(dinvr_t[:], dinvr_in[:])
                iota_t = cpool.tile([P, P], BF16)
                nc.sync.dma_start(iota_t[:], iota_in[:])
                s2i_t = cpool.tile([P, P], BF16)
                nc.sync.dma_start(s2i_t[:], s2i_in[:])
                w_t = cpool.tile([P, P], BF16)
                nc.sync.dma_start(w_t[:], w_in[:])
                b_t = cpool.tile([P, 1], F32)
                nc.sync.dma_start(b_t[:], b_in[:])
                const_tiles.update(
                    dst_t=dst_t, dinvr_t=dinvr_t, iota_t=iota_t,
                    s2i_t=s2i_t, w_t=w_t, b_t=b_t,
                )

            # trace-time ring bookkeeping
            g_of_tile = {}  # global tile idx -> (tile_obj, col)
            s_of_inc = {}  # incidence idx -> (tile_obj, col)
            next_call = [0] * CHUNKS
            covered = [0] * CHUNKS  # slots issued per chunk (stream-local)
            built_inc = 0
            qctr = 0

            def issue_call(c):
                nonlocal qctr
                s0, nidx = calls_by_chunk[c][next_call[c]]
                nt = (nidx + P - 1) // P
                it = ipool.tile([P, nidx // 16], I16, tag="idx")
                nc.sync.dma_start(it[:], idx_in[:, s0 // 16 : (s0 + nidx) // 16])
                g = gpool.tile([P, nt, P], BF16, tag="g")
                nc.gpsimd.dma_gather(
                    g[:],
                    xt[c * CHR : (c + 1) * CHR, :],
                    it[:],
                    nidx,
                    nidx,
                    P,
                    queue_num=qctr % NQUEUES,
                )
                qctr += 1
                t0 = s0 // P
                for k in range(nt):
                    g_of_tile[t0 + k] = (g, k)
                next_call[c] += 1
                covered[c] += nidx
                load_consts()

            def build_s_upto(i_hi):
                nonlocal built_inc
                while built_inc < i_hi:
                    nb = min(SBATCH, n_inc - built_inc)
                    s = spool.tile([P, nb, P], BF16, tag="s")
                    dst_t = const_tiles["dst_t"]
                    iota_t = const_tiles["iota_t"]
                    nc.vector.tensor_tensor(
                        s[:],
                        iota_t[:, None, :].to_broadcast([P, nb, P]),
                        dst_t[:, built_inc : built_inc + nb, None].to_broadcast(
                            [P, nb, P]
                        ),
                        mybir.AluOpType.is_equal,
                    )
                    for k in range(nb):
                        s_of_inc[built_inc + k] = (s, k)
                    built_inc += nb

            # stream-local end slot of each window's bin per chunk, and the
            # max incidence index needed per window
            bin_end_local = np.zeros((NWIN, CHUNKS), np.int64)
            for c in range(CHUNKS):



                bin_end_local[:, c] = np.cumsum(sz16[:, c])
            inc_end = np.zeros(NWIN, np.int64)
            hi = 0
            for w in range(NWIN):
                for c in range(CHUNKS):
                    for t, i in inc_map[w][c]:
                        hi = max(hi, i + 1)
                inc_end[w] = hi

            def finish_window(ps, w):
                # evac -> W -> bias/ReLU -> store; emitted one window behind
                # the aggregation so the PE never stalls on the DVE evac
                dinvr_t = const_tiles["dinvr_t"]
                w_t = const_tiles["w_t"]
                b_t = const_tiles["b_t"]
                m = mpool.tile([P, P], BF16, tag="m")
                nc.vector.tensor_tensor(
                    m[:], ps[:], dinvr_t[:, w * P : (w + 1) * P],
                    mybir.AluOpType.mult,
                )
                ph = psh_pool.tile([P, P], F32, tag="psh")
                nc.tensor.matmul(ph[:], w_t[:], m[:], start=True, stop=True)
                xo_sb = xopool.tile([P, P], BF16, tag="xo")
                nc.scalar.activation(xo_sb[:], ph[:], AF.Relu, bias=b_t[:])
                nc.sync.dma_start(xo[:, w * P : (w + 1) * P], xo_sb[:])

            pending = None
            for w in range(NWIN):
                wa = min(w + LOOKAHEAD, NWIN - 1)
                for c in range(CHUNKS):
                    while (
                        next_call[c] < len(calls_by_chunk[c])
                        and covered[c] < bin_end_local[wa, c]
                    ):
                        issue_call(c)
                build_s_upto(int(inc_end[min(w + 5, NWIN - 1)]))

                s2i_t = const_tiles["s2i_t"]
                ps = psm_pool.tile([P, P], F32, tag="psm")
                first = True
                for c in range(CHUNKS):
                    for t, i in inc_map[w][c]:
                        g, gk = g_of_tile[t]
                        s, sk = s_of_inc[i]
                        nc.tensor.matmul(
                            ps[:], g[:, gk, :], s[:, sk, :],
                            start=first, stop=False,
                        )
                        first = False
                gs = selfpool.tile([P, P], BF16, tag="gself")
                nc.sync.dma_start(gs[:], xt_own[w * P : (w + 1) * P, :])
                nc.tensor.matmul(ps[:], gs[:], s2i_t[:], start=first, stop=True)

                if pending is not None:
                    finish_window(*pending)
                pending = (ps, w)
            finish_window(*pending)
    nc.compile()
    return nc


_CACHE = {}


def _get_program(src, dst):
    key = (hash(src.tobytes()) ^ hash(dst.tobytes()), len(src))
    if key not in _CACHE:
        sz16, calls, inc_map, nt_total, n_inc, per_core, assign = _host_prep(src, dst)
        nc = _build_program(sz16, calls, inc_map, nt_total, n_inc)
        # node_at_pos[v] = original node id occupying virtual position v
        node_at_pos = np.empty(N_NODES, np.int64)
        node_at_pos[assign] = np.arange(N_NODES)
        _CACHE.clear()
        _CACHE[key] = (nc, per_core, node_at_pos)
    return _CACHE[key]


def kernel(
    x,
    edge_index,
    batch,
    batch_size,
    conv_w,
    conv_b,
    fc1_w,
    fc1_b,
    fc2_w,
    fc2_b,
    profile=False,
):
    x = np.asarray(x, np.float32)
    edge_index = np.asarray(edge_index, np.int64)
    batch = np.asarray(batch, np.int64)
    conv_w = np.asarray(conv_w, np.float32)
    conv_b = np.asarray(conv_b, np.float32)
    G = int(batch_size)
    n = x.shape[0]
    assert n == N_NODES and edge_index.shape[0] == 2

    src, dst = edge_index[0], edge_index[1]
    deg = np.bincount(dst, minlength=n).astype(np.float32) + 2.0
    dinv = (1.0 / np.sqrt(deg)).astype(np.float32)

    nc, per_core, node_at_pos = _get_program(src, dst)

    iota = np.tile(np.arange(P, dtype=np.float32), (P, 1)).astype(NPBF16)
    s2i = (2.0 * np.eye(P, dtype=np.float32)).astype(NPBF16)
    dinv_v = dinv[node_at_pos]  # dinv by virtual position
    dinv_rows = []
    for c in range(N_CORES):
        dr = np.zeros(NPC_PAD, np.float32)
        dr[:NPC] = dinv_v[c * NPC : (c + 1) * NPC]
        dinv_rows.append(np.tile(dr[None, :], (P, 1)).astype(NPBF16))

    xt = (dinv[:, None] * x).astype(NPBF16)
    total_ns = 0
    for layer in range(N_CONVS):
        wmat_bf = conv_w[layer].astype(NPBF16)
        bias_f = conv_b[layer].astype(np.float32).reshape(P, 1)
        xt_v = xt[node_at_pos]  # virtual-position-ordered rows for self loops
        maps = []
        for c in range(N_CORES):
            own = np.zeros((NPC_PAD, P), NPBF16)
            own[:NPC] = xt_v[c * NPC : (c + 1) * NPC]
            maps.append(
                {
                    "xt": xt,
                    "xt_own": own,
                    "idx": per_core[c]["idx"],
                    "dstloc": per_core[c]["dstloc"],
                    "wmat": wmat_bf,
                    "bias": bias_f,
                    "dinv_row": dinv_rows[c],
                    "iota": iota,
                    "s2i": s2i,
                }
            )
        res = run_bass_kernel_spmd(
            nc, maps, core_ids=list(range(N_CORES)), trace=profile
        )
        if profile and res.exec_time_ns is not None:
            total_ns += int(res.exec_time_ns)
        xp = np.empty((n, P), np.float32)
        for c in range(N_CORES):
            blk = res.results[c]["xo"].astype(np.float32).T
            xp[node_at_pos[c * NPC : (c + 1) * NPC]] = blk[:NPC]
        if layer < N_CONVS - 1:
            xt = (dinv[:, None] * xp).astype(NPBF16)

    starts = np.searchsorted(batch, np.arange(G))
    sums = np.add.reduceat(xp, starts, axis=0)
    cnt = np.bincount(batch, minlength=G).astype(np.float32)
    sums[cnt == 0] = 0.0
    pooled = sums / np.maximum(cnt, 1.0)[:, None]
    h = np.maximum(
        pooled @ np.asarray(fc1_w, np.float32) + np.asarray(fc1_b, np.float32), 0.0
    )
    out = h @ np.asarray(fc2_w, np.float32) + np.asarray(fc2_b, np.float32)
    if profile:
        print(f"HW exec time: {total_ns} ns")
    return out[:, 0].astype(np.float32)
